# revision 16
# baseline (speedup 1.0000x reference)
"""Attention pooling (segment softmax + weighted segment-mean) on 8 Trainium2 cores.

Reference computation (per full input):
    logits = leaky_relu(feature @ a, 0.2)                    # [N]
    att    = segment_softmax(logits, batch)                  # [N]
    out    = segment_sum(att[:, None] * feature) / counts    # [1024, 256]

Strategy: batch ids are sorted, so core c owns the 128 contiguous segments
[128c, 128c+128), split into 4 groups of 32 segments. Each group's nodes
are padded to whole 128-node subtiles; per-group subtile caps are the max
over the 8 cores so one SPMD program fits all. The projection vector `a`
is FOLDED INTO THE FEATURES host-side (F' = F * diag(a)): a diagonal
scaling commutes through the weighted segment-sum, so the device logits
are plain row-sums of F' and the host divides the output columns by a at
the end. Features ship as fp16 [f'|1] rows (the literal 1.0 feeds the
softmax denominator through the same matmul) laid out so each batch of 16
subtiles is one [128 x 8224B] DMA slab split across two rings (4KB
packets, ~150+ GB/s per ring). One-hot segment indicators are precomputed
host-side and DMA'd as fp8.

Per batch of 16 subtiles (2048 nodes):
  - ACT: z[j] = accum(F'_j) for subtiles 0-3 (Copy + accum_out),
  - DVE: z[4:16] via add-tree (128+128 -> 64+64 -> reduce),
  - DVE: leaky = max(z, 0.2 z); ACT: ex = exp(leaky - 6) in fp16,
  - DVE: W = onehot_fp8 * ex (broadcast) -> [128, 16, 32] fp16,
  - PE: [sums | denom] += W_j.T @ [F'_j | 1], chained into the PSUM rows
    [32g, 32g+32) of the subtile's group (tile_position 32g).
A two-stage software pipeline (leaky/exp one batch behind, W/matmul two
behind) plus 4-deep DMA prefetch keeps every engine queue unblocked.
The softmax max-subtraction is replaced by a constant shift (-6): sums
and denom scale identically so the ratio is unchanged. Counts and the
final (sums / denom / counts / a) normalization are O(segments) on host.
"""

from contextlib import ExitStack

import numpy as np

import concourse.bacc as bacc
import concourse.tile as tile
from concourse import mybir
from concourse.bass_utils import run_bass_kernel_spmd

N_CORES = 8
P = 128                 # partitions / nodes per subtile
H = 256                 # hidden
NSEG = 1024
SEG = NSEG // N_CORES   # 128 segments per core
GSEG = 32               # segments per group
NGRP = SEG // GSEG      # 4 groups per core
BS = 16                 # subtiles per batch (2048 nodes)
HP1 = H + 1
HP2 = H + 2            # row pitch: [f | 1 | 0], even length keeps 4B alignment
EXP_SHIFT = -6.0
NEG_SLOPE = 0.2
N_ACT = 4               # subtiles of each batch reduced on ACT (rest on DVE)

_FEAT, _ISEG, _OUT = "feat", "iseg", "out"
F8 = mybir.dt.float8e4
F16 = mybir.dt.float16
F32 = mybir.dt.float32


def _build_program(nt, grp, start, stop):
    """grp/start/stop: per-subtile group id and PSUM chain start/stop flags."""
    nb = -(-nt // BS)
    nc = bacc.Bacc("TRN2", target_bir_lowering=False, debug=False)
    feat_d = nc.dram_tensor(_FEAT, [nb * P, BS * HP2], F16, kind="ExternalInput").ap()
    iseg_d = nc.dram_tensor(_ISEG, [P, nt * GSEG], F8, kind="ExternalInput").ap()
    out_d = nc.dram_tensor(_OUT, [SEG, HP1], F32, kind="ExternalOutput").ap()
    feat_r = feat_d.rearrange("(b p) (k h) -> b p k h", p=P, k=BS)

    with tile.TileContext(nc) as tc, ExitStack() as ctx:
        consts = ctx.enter_context(tc.tile_pool(name="consts", bufs=1))
        fpool = ctx.enter_context(tc.tile_pool(name="f", bufs=11))
        ipool = ctx.enter_context(tc.tile_pool(name="iseg", bufs=11))
        zpool = ctx.enter_context(tc.tile_pool(name="z", bufs=16))
        wpool = ctx.enter_context(tc.tile_pool(name="w", bufs=6))
        tpool = ctx.enter_context(tc.tile_pool(name="tree", bufs=5))
        opool = ctx.enter_context(tc.tile_pool(name="o", bufs=1))
        psum = ctx.enter_context(tc.tile_pool(name="psum", bufs=1, space="PSUM"))

        c02_sb = consts.tile([P, BS], F32)
        shift_sb = consts.tile([P, 1], F32)
        nc.vector.memset(c02_sb, NEG_SLOPE)
        nc.vector.memset(shift_sb, EXP_SHIFT)

        acc = psum.tile([SEG, HP1], F32, tag="acc")

        def bsz(q):                      # subtiles in batch q (ragged tail)
            return min(BS, nt - q * BS)

        def emit_w_and_matmul(q, F, I, ex):
            n = bsz(q)
            W = wpool.tile([P, n, GSEG], F16)
            nc.vector.tensor_tensor(
                out=W, in0=I[:, 0:n, :],
                in1=ex[:, 0:n, None].broadcast_to([P, n, GSEG]),
                op=mybir.AluOpType.mult)
            for j in range(n):
                t = q * BS + j
                g = grp[t]
                nc.tensor.matmul(acc[g * GSEG:(g + 1) * GSEG, :],
                                 lhsT=W[:, j, :], rhs=F[:, j, 0:HP1],
                                 start=start[t], stop=stop[t],
                                 tile_position=(0, g * GSEG))

        def finish_z(st):
            (qp, Fp, Ip, zp) = st
            n = bsz(qp)
            tl = zpool.tile([P, n], F32, tag="t")
            nc.vector.tensor_tensor(out=tl, in0=zp[:, 0:n], in1=c02_sb[:, 0:n],
                                    op=mybir.AluOpType.mult)
            ll = zpool.tile([P, n], F32, tag="l")
            nc.vector.tensor_tensor(out=ll, in0=tl, in1=zp[:, 0:n],
                                    op=mybir.AluOpType.max)
            ex = zpool.tile([P, n], F16, tag="ex")
            nc.scalar.activation(ex, ll, mybir.ActivationFunctionType.Exp,
                                 bias=shift_sb[:, :])
            return (qp, Fp, Ip, ex)

        def issue_dma(q):
            F = fpool.tile([P, BS, HP2], F16)
            nc.sync.dma_start(F[:, 0:BS // 2, :], feat_r[q, :, 0:BS // 2, :])
            nc.gpsimd.dma_start(F[:, BS // 2:BS, :], feat_r[q, :, BS // 2:BS, :])
            I = ipool.tile([P, BS, GSEG], F8)
            nc.gpsimd.dma_start(
                I[:, 0:bsz(q), :],
                iseg_d[:, q * BS * GSEG:(q * BS + bsz(q)) * GSEG]
                .rearrange("p (k g) -> p k g", g=GSEG))
            return (F, I)

        PREFETCH = 7
        loaded = [issue_dma(q) for q in range(min(PREFETCH, nb))]
        stage1 = None   # (q, F, I, z)  awaiting leaky -> exp
        stage2 = None   # (q, F, I, ex) awaiting W + matmul
        for q in range(nb):
            if q + PREFETCH < nb:
                loaded.append(issue_dma(q + PREFETCH))
            F, I = loaded[q]
            n = bsz(q)

            if stage2 is not None:
                emit_w_and_matmul(*stage2)
                stage2 = None
            if stage1 is not None:      # leaky(q-1) on DVE
                (qp, Fp, Ip, zp) = stage1
                np_ = bsz(qp)
                tl = zpool.tile([P, np_], F32, tag="t")
                nc.vector.tensor_tensor(out=tl, in0=zp[:, 0:np_],
                                        in1=c02_sb[:, 0:np_],
                                        op=mybir.AluOpType.mult)
                ll = zpool.tile([P, np_], F32, tag="l")
                nc.vector.tensor_tensor(out=ll, in0=tl, in1=zp[:, 0:np_],
                                        op=mybir.AluOpType.max)
                stage1 = (qp, Fp, Ip, ll)

            # z = row-sums of F' (a folded in host-side). ACT reduces
            # subtiles [0, N_ACT) via Copy+accum, DVE an add-tree for the
            # rest. Slices stop at H so the ones column stays out of z.
            z = zpool.tile([P, BS], F32, tag="z")
            na = min(N_ACT, n)
            for j in range(na):
                dump = tpool.tile([P, H], F16, tag="dump")
                nc.scalar.activation(dump, F[:, j, 0:H],
                                     mybir.ActivationFunctionType.Copy,
                                     accum_out=z[:, j:j + 1])
            nd = n - na
            if nd > 0:
                t1 = tpool.tile([P, nd, 128], F16, tag="t1")
                nc.vector.tensor_tensor(out=t1, in0=F[:, na:n, 0:128],
                                        in1=F[:, na:n, 128:256],
                                        op=mybir.AluOpType.add)
                t2 = tpool.tile([P, nd, 64], F16, tag="t2")
                nc.vector.tensor_tensor(out=t2, in0=t1[:, :, 0:64],
                                        in1=t1[:, :, 64:128],
                                        op=mybir.AluOpType.add)
                t3 = tpool.tile([P, nd, 32], F16, tag="t3")
                nc.vector.tensor_tensor(out=t3, in0=t2[:, :, 0:32],
                                        in1=t2[:, :, 32:64],
                                        op=mybir.AluOpType.add)
                nc.vector.tensor_reduce(out=z[:, na:n], in_=t3,
                                        axis=mybir.AxisListType.X,
                                        op=mybir.AluOpType.add)

            if stage1 is not None:      # exp(q-1) on ACT, after accums(q)
                (qp, Fp, Ip, ll) = stage1
                ex = zpool.tile([P, bsz(qp)], F16, tag="ex")
                nc.scalar.activation(ex, ll, mybir.ActivationFunctionType.Exp,
                                     bias=shift_sb[:, :])
                stage2 = (qp, Fp, Ip, ex)
            stage1 = (q, F, I, z)

        if stage2 is not None:
            emit_w_and_matmul(*stage2)
        emit_w_and_matmul(*finish_z(stage1))

        out_sb = opool.tile([SEG, HP1], F32)
        nc.vector.tensor_copy(out_sb, acc)
        nc.sync.dma_start(out_d, out_sb)

    nc.compile()
    return nc


def kernel(feature, a, batch, _trace=False):
    feature = np.asarray(feature, dtype=np.float32)
    a = np.asarray(a, dtype=np.float32)
    batch = np.asarray(batch)
    n = feature.shape[0]
    assert feature.shape == (n, H) and batch.shape == (n,)

    gb = np.searchsorted(batch, np.arange(0, NSEG + 1, GSEG))
    gcnt = np.diff(gb).reshape(N_CORES, NGRP)          # [core, group] counts
    gcap = np.maximum(-(-gcnt // P), 1).max(axis=0)    # subtiles/group (shared)
    nt = int(gcap.sum())
    nb = -(-nt // BS)
    gcap[NGRP - 1] += nb * BS - nt                     # absorb tail padding
    nt = nb * BS
    gstart = np.concatenate([[0], np.cumsum(gcap)])    # subtile offsets

    grp = np.empty(nt, dtype=np.int64)
    start = np.zeros(nt, dtype=bool)
    stop = np.zeros(nt, dtype=bool)
    for g in range(NGRP):
        grp[gstart[g]:gstart[g + 1]] = g
        start[gstart[g]] = True
        stop[gstart[g + 1] - 1] = True

    avec = a.reshape(-1)
    feat16 = (feature * avec[None, :]).astype(np.float16)
    f8 = mybir.dt.np(F8)

    in_maps = []
    for c in range(N_CORES):
        flat = np.zeros((nt * P, HP2), dtype=np.float16)   # (subtile, p) order
        segflat = np.full(nt * P, GSEG, dtype=np.int32)    # pad: no match
        for g in range(NGRP):
            gi = c * NGRP + g
            s0, e0 = int(gb[gi]), int(gb[gi + 1])
            cnt = e0 - s0
            base = int(gstart[g]) * P
            assert cnt <= int(gcap[g]) * P, (c, g, cnt)
            flat[base:base + cnt, 0:H] = feat16[s0:e0]
            flat[base:base + cnt, H] = 1.0
            segflat[base:base + cnt] = batch[s0:e0] - (c * SEG + g * GSEG)
        # DMA layout [b, p, j, :]: batch b row p holds subtiles 16b..16b+15
        buf = np.ascontiguousarray(
            flat.reshape(nb, BS, P, HP2).transpose(0, 2, 1, 3))
        segsub = segflat.reshape(nt, P)
        onehot = (segsub[:, :, None] == np.arange(GSEG)[None, None, :])
        iseg = np.ascontiguousarray(
            onehot.transpose(1, 0, 2).reshape(P, nt * GSEG).astype(f8))
        in_maps.append({
            _FEAT: buf.reshape(nb * P, BS * HP2),
            _ISEG: iseg,
        })

    nc = _build_program(nt, grp.tolist(),
                        [bool(x) for x in start], [bool(x) for x in stop])
    res = run_bass_kernel_spmd(nc, in_maps, core_ids=list(range(N_CORES)),
                               trace=_trace)

    counts = np.bincount(batch.astype(np.int64), minlength=NSEG).astype(np.float32)
    counts = np.maximum(counts, 1.0)
    out = np.zeros((NSEG, H), dtype=np.float32)
    for c in range(N_CORES):
        blk = res.results[c][_OUT]          # [128, 257]
        sums, denom = blk[:, :H], blk[:, H]
        seg0 = c * SEG
        safe = np.maximum(denom, 1e-30)[:, None]
        out[seg0:seg0 + SEG] = np.where(
            denom[:, None] > 0.0,
            sums / safe / counts[seg0:seg0 + SEG, None] / avec[None, :],
            0.0,
        )
    if _trace:
        kernel.last_results = res
    return out
